# revision 26
# baseline (speedup 1.0000x reference)
"""MoE (16 experts, top-1 gate, D=H=768) Trainium2 kernel.

Strategy (expert-parallel, per the sharding hint):
  - Host computes the gate (logits argmax) — this IS the dispatch step that
    decides the sharding: tokens are routed to the core owning their expert.
  - 16 experts are sharded 2-per-core across the 8 NeuronCores. Experts are
    sorted by routed-token count: the 8 largest go in slot 0 (capacity C0),
    the 8 smallest in slot 1 (capacity C1 <= C0), so every core does the
    identical padded work and padding waste is minimized. Capacities are
    rounded to 32 columns (the matmul free dim has no 128 constraint).
  - Each core runs the two-GEMM MLP (x @ W1.T -> GELU -> @ W2.T) for its two
    experts over its routed tokens, padded to the slot capacity.
  - Host scatters per-token outputs back to the full [B, N, D] tensor.

Device kernel details:
  - Matmul operands are fp16 (PE full rate + FWL weight loads; fp32
    LDWEIGHTS cannot pipeline and halves matmul throughput; fp16 has 10
    mantissa bits -> rel err ~4e-4 end to end). PSUM accumulation is fp32,
    biases/GELU applied on fp32 PSUM. Outputs are written back fp16 (host
    converts) to halve output HBM traffic.
  - A few raw (non-Tile) dummy matmuls on an uninitialized scratch tile are
    emitted BEFORE the TileContext, so they execute during the fixed engine
    boot preamble. They keep the PE busy so its HAM clock gate (cold
    1.2 GHz -> warm 2.4 GHz after ~3.4 us of sustained activity)
    un-throttles before the real matmul stream begins.
  - DMA ring assignment: input pieces are interleaved across BOTH HWDGE
    rings (SP and ACT) in matmul consumption order — the rings share the
    ~335 GB/s HBM budget unevenly, so splitting every tensor across both
    bounds the arrival time of each phase's working set. The ACT ring gets
    only four input pieces so the scalar engine is free for GELU early.
    Outputs ride the SP ring (idle after the input fill; SWDGE/gpsimd
    output DMAs measured ~10 us late — DVE 16-bit 2-port mode starves the
    Q7 descriptor rings), except the final per-d-chunk outputs which
    alternate SP/ACT so their descriptor generation overlaps. Biases ride
    the GPSIMD SWDGE ring (issued at t~7 us, landing well before the first
    GELU, ahead of any DVE activity).
  - BIR post-processing: the first piece of w1 slot 0 (SP) and of x slot 0
    (ACT) is hoisted into 'main' ahead of the engines' entry rendezvous so
    data starts streaming ~1.5 us earlier; the unused bass const-AP memsets
    (and optionally the init barrier) are stripped.
"""

import json

import ml_dtypes
import numpy as np

import concourse.bass as bass
import concourse.mybir as mybir
import concourse.tile as tile
from concourse.bass_utils import run_bass_kernel_spmd

E = 16          # experts
D = 768         # d_model
H = 768         # d_hidden
NCORES = 8
EPC = E // NCORES   # experts (slots) per core = 2
DC = D // 128       # 6 d-chunks
HC = H // 128       # 6 h-chunks

MM_DTYPE = "f16"   # "f16" | "bf16" | "f32r"
N_WARM = 7          # warm-up matmuls (HAM un-throttle), N=512 each
HOIST_LIMITS = {"SP": 1, "Activation": 1}
STRIP_CONST_INIT = True
STRIP_INIT_BARRIER = True

F32 = mybir.dt.float32
F16 = mybir.dt.float16


def _mm_dt():
    if MM_DTYPE == "f16":
        # fp16 runs at the same PE rate as bf16 (1 col/cycle + FWL weight
        # loads) but has 10 mantissa bits instead of 7 — ~6x lower rounding
        # error. All operands here (|x| < ~6, |W| < ~0.2, GELU outputs) are
        # far inside fp16 range and accumulation is fp32 PSUM.
        return mybir.dt.float16, np.float16
    if MM_DTYPE == "bf16":
        return mybir.dt.bfloat16, ml_dtypes.bfloat16
    return mybir.dt.float32r, np.float32


def _split_multi_waits(bir):
    """Walrus (this image's build) rejects >1 sem-wait on one instruction
    ("Too many sync wait commands" on the TileContext-exit Drain). Move
    excess waits onto a chain of same-engine NoOps directly before the
    instruction — the sequencer runs them in program order, so the
    happens-after relation is preserved exactly."""
    nid = 0
    for fn in bir["functions"]:
        for blk in fn["blocks"]:
            out = []
            for ins in blk["instructions"]:
                si = ins.get("sync_info")
                waits = (si or {}).get("on_wait") or []
                if len(waits) > 1:
                    for w in waits[:-1]:
                        nid += 1
                        out.append({
                            "debug": ins.get("debug", 0),
                            "name": f"I-waitfix{nid}",
                            "opcode": "NoOp",
                            "engine": ins["engine"],
                            "ins": [],
                            "outs": [],
                            "sync_info": {"on_update": [], "on_wait": [w]},
                        })
                    si["on_wait"] = waits[-1:]
                out.append(ins)
            blk["instructions"] = out
    return bir


def _main_and_tile(bir):
    for fn in bir["functions"]:
        blocks = {b["name"]: b for b in fn["blocks"]}
        main = blocks.get("main")
        tbs = [b for n, b in blocks.items()
               if n != "main" and not n.endswith("_end")]
        if main is not None and len(tbs) == 1:
            yield main, tbs[0]


def _hoist_input_dmas(bir, limits):
    """Move the first `limits[engine]` wait-free DMACopy instructions from
    the tile block into 'main', directly before that engine's first Drain
    (the entry rendezvous arrive), so the first input pieces stream during
    part of the fixed boot preamble. Per-engine program order (hence DMA
    ring order / semaphore accounting) is preserved exactly."""
    for main, tb in _main_and_tile(bir):
        budget = dict(limits)
        hoisted, kept = [], []
        for ins in tb["instructions"]:
            si = ins.get("sync_info") or {}
            if (ins["opcode"] == "DMACopy" and not si.get("on_wait")
                    and budget.get(ins["engine"], 0) > 0):
                budget[ins["engine"]] -= 1
                hoisted.append(ins)
            else:
                kept.append(ins)
        if not hoisted:
            continue
        out, done = [], set()
        for ins in main["instructions"]:
            if ins["opcode"] == "Drain" and ins["engine"] not in done:
                done.add(ins["engine"])
                out.extend(h for h in hoisted if h["engine"] == ins["engine"])
            out.append(ins)
        if [h for h in hoisted if h["engine"] not in done]:
            continue  # unexpected engine: leave BIR unmodified for safety
        tb["instructions"] = kept
        main["instructions"] = out
    return bir


def _strip_const_init(bir, strip_barrier):
    """Remove the bass const-AP memsets (f32 0/1, bf16 1, u8 127 — unused by
    this kernel) and optionally the init all-engine barrier from 'main', so
    engines reach their first useful instruction sooner. The kernel's own
    cross-engine ordering is entirely semaphore-based (Tile-generated), so
    the barrier is not needed for correctness here."""
    for main, _tb in _main_and_tile(bir):
        out = []
        for ins in main["instructions"]:
            if ins["opcode"] == "Memset" and any(
                    "const-" in str(o.get("name", "")) for o in ins.get("outs", [])):
                continue
            if strip_barrier and (
                    "barrier_" in ins.get("name", "")
                    or (ins["opcode"] == "Drain" and _is_barrier_drain(ins))):
                continue
            out.append(ins)
        main["instructions"] = out
    return bir


def _is_barrier_drain(ins):
    si = ins.get("sync_info") or {}
    for grp in (si.get("on_wait") or []) + (si.get("on_update") or []):
        if "barrier_" in str(grp.get("ant_name", "")):
            return True
    return False


def _finalize(nc):
    bir = json.loads(nc.to_json_bytes())
    bir = _split_multi_waits(bir)
    bir = _hoist_input_dmas(bir, HOIST_LIMITS)
    if STRIP_CONST_INIT:
        bir = _strip_const_init(bir, STRIP_INIT_BARRIER)
    data = json.dumps(bir).encode()
    nc.to_json_bytes = lambda: data
    return nc


def _chunking(C):
    chunks = []
    c0 = 0
    while c0 < C:
        cw = min(512, C - c0)
        chunks.append((c0, cw))
        c0 += cw
    return chunks


def _build(C0, C1):
    """Per-core SPMD kernel: slot 0 with token capacity C0, slot 1 with C1
    (both multiples of 32). Token dim in chunks of <=512 (PSUM bank limit
    for fp32 accumulation)."""
    caps = [C0, C1]
    slot_chunks = [_chunking(C) for C in caps]

    MMDT, _ = _mm_dt()

    nc = bass.Bass("TRN2", target_bir_lowering=False, debug=False,
                   num_devices=NCORES)
    xts_d = [nc.dram_tensor(f"xt{s}", [128, DC, caps[s]], MMDT,
                            kind="ExternalInput") for s in range(EPC)]
    yts_d = [nc.dram_tensor(f"yt{s}", [128, DC, caps[s]], F16,
                            kind="ExternalOutput") for s in range(EPC)]
    w1t = nc.dram_tensor("w1t", [EPC, 128, DC, H], MMDT, kind="ExternalInput")
    w2t = nc.dram_tensor("w2t", [EPC, 128, HC, D], MMDT, kind="ExternalInput")
    # biases packed into one [128, EPC*(HC+DC)] f32 tensor: per slot s the
    # columns are [b1 cols (HC), b2 cols (DC)].
    ball = nc.dram_tensor("ball", [128, EPC * (HC + DC)], F32,
                          kind="ExternalInput")

    # scratch SBUF for the HAM warm-up matmuls (read uninitialized)
    warm_sb = nc.alloc_sbuf_tensor("warm_sb", [128, 512], MMDT)

    GELU = mybir.ActivationFunctionType.Gelu

    with tile.TileContext(nc) as tc:
        with (
            tc.tile_pool(name="xp", bufs=1) as xp,
            tc.tile_pool(name="wp", bufs=1) as wp,
            tc.tile_pool(name="gp", bufs=2) as gp,
            tc.tile_pool(name="yp", bufs=4) as yp,
            tc.tile_pool(name="bp", bufs=1) as bp,
            tc.tile_pool(name="pp", bufs=2, space="PSUM") as pp,
        ):
            # ---- HAM warm-up: matmuls on uninitialized scratch, rotated
            # through the MM2 PSUM ring (tag ps2) before MM2 ever uses it.
            # high_priority pins them to the front of the PE queue, so the
            # PE is busy (and the HAM clock gate un-throttles) while the
            # input DMAs stream.
            with tc.high_priority():
                for _ in range(N_WARM):
                    wps = pp.tile([128, 512], F32, tag="ps2", name="wps")
                    nc.tensor.matmul(wps[:, :], warm_sb.ap()[:, 0:128],
                                     warm_sb.ap(), start=True, stop=True)

            # ---- phase 1: issue ALL input DMAs. No compute-dependent wait
            # ever enters any input ring, so they stream continuously.
            tiles = []
            for s in range(EPC):
                w1s = wp.tile([128, DC, H], MMDT, tag=f"w1_{s}",
                              name=f"w1s_{s}")
                w2s = wp.tile([128, HC, D], MMDT, tag=f"w2_{s}",
                              name=f"w2s_{s}")
                xcs = [xp.tile([128, DC, 512], MMDT, tag=f"x_{s}_{ci}",
                               name=f"xc_{s}_{ci}")
                       for ci in range(len(slot_chunks[s]))]
                tiles.append((w1s, w2s, xcs))
            bt = bp.tile([128, EPC * (HC + DC)], F32, tag="b", name="bt")

            (w1s0, w2s0, xcs0), (w1s1, w2s1, xcs1) = tiles
            xc0, xc1 = xcs0[0], xcs1[0]
            cw0 = slot_chunks[0][0][1]
            cw1 = slot_chunks[1][0][1]
            # Interleave the slot-0 working set (w1s0 + x chunk0) across
            # both rings in 2-dc pieces, in matmul consumption order (the
            # dc-major first GEMM consumes (w1[dc], x[dc]) pairs). Finer
            # per-dc pieces measured ~40% lower ring throughput. The first
            # piece on each ring is hoisted into 'main' by _finalize.
            # Three input channels: SP + ACT HWDGE rings, plus the GPSIMD
            # SWDGE ring (idle and conflict-free before the DVE's 16-bit
            # 2-port mode starts at the first bias-add, ~18us in). w1s0 is
            # split across SP+ACT so no channel carries a >0.8MB long pole;
            # x slot0 rides SWDGE. Pieces land in dc-major consumption
            # order: dc0 (SP+ACT, first piece each hoisted into 'main'),
            # dc1-3 (SP w1 + GPS x), dc4-5 (ACT w1 + GPS x).
            nc.gpsimd.dma_start(bt[:, :], ball.ap())   # biases first on GPS
            nc.sync.dma_start(w1s0[:, 0:1], w1t.ap()[0, :, 0:1])          # SP*
            nc.scalar.dma_start(xc0[:, 0:1, :cw0],
                                xts_d[0].ap()[:, 0:1, 0:cw0])             # ACT*
            nc.sync.dma_start(w1s0[:, 1:4], w1t.ap()[0, :, 1:4])
            nc.gpsimd.dma_start(xc0[:, 1:4, :cw0],
                                xts_d[0].ap()[:, 1:4, 0:cw0])
            nc.scalar.dma_start(w1s0[:, 4:6], w1t.ap()[0, :, 4:6])
            nc.gpsimd.dma_start(xc0[:, 4:6, :cw0],
                                xts_d[0].ap()[:, 4:6, 0:cw0])
            # slot-1 closure next (its first GEMM runs before either second
            # GEMM, hiding the slot-0 GELU drain and the w2 wait).
            nc.sync.dma_start(w1s1[:, 0:3], w1t.ap()[1, :, 0:3])
            nc.scalar.dma_start(xc1[:, :, :cw1], xts_d[1].ap()[:, :, 0:cw1])
            nc.gpsimd.dma_start(w1s1[:, 3:6], w1t.ap()[1, :, 3:6])
            # w2 split across BOTH rings by OUTPUT d-chunks (MM2's group
            # for output dc needs w2s[:, all hc, dc*128:...]); a single
            # ring cannot deliver a w2 in time.
            nc.sync.dma_start(w2s0[:, :, 0:3 * 128], w2t.ap()[0, :, :, 0:384])
            nc.scalar.dma_start(w2s0[:, :, 3 * 128:], w2t.ap()[0, :, :, 384:768])
            nc.sync.dma_start(w2s1[:, :, 0:3 * 128], w2t.ap()[1, :, :, 0:384])
            nc.scalar.dma_start(w2s1[:, :, 3 * 128:], w2t.ap()[1, :, :, 384:768])
            # any extra x chunks (caps > 512; not hit for this problem size)
            for s, xcs in ((0, xcs0), (1, xcs1)):
                for ci, (c0, cw) in enumerate(slot_chunks[s]):
                    if ci == 0:
                        continue
                    eng = nc.sync if (ci % 2) else nc.scalar
                    eng.dma_start(xcs[ci][:, :, :cw],
                                  xts_d[s].ap()[:, :, c0:c0 + cw])

            # ---- phase 2: compute.
            # Order: MM1(s0), MM1(s1), MM2(s0), MM2(s1) — slot 1's first
            # GEMM fills the PE bubble where slot 0's GELU drain and w2
            # arrival would otherwise stall the FIFO.
            def b1col(s, hc):
                return bt[:, s * (HC + DC) + hc: s * (HC + DC) + hc + 1]

            def b2col(s, dc):
                base = s * (HC + DC) + HC
                return bt[:, base + dc: base + dc + 1]

            def mm1(s, ci, dc_major):
                c0, cw = slot_chunks[s][ci]
                w1s = tiles[s][0]
                xc = tiles[s][2][ci]
                gc = gp.tile([128, HC, 512], MMDT, tag="g",
                             name=f"gc_{s}_{ci}")
                if dc_major:
                    # 6 accumulation groups open at once (6 PSUM banks):
                    # each arriving (w1[dc], x[dc]) pair unlocks 6 matmuls,
                    # so the PE streams during the HBM fill instead of
                    # FIFO-stalling on the first hc-group's full d sweep.
                    # The final dc pass closes groups one hc at a time with
                    # the GELU fused right after each close, so the ACT
                    # drain overlaps the remaining closes and the PSUM ring
                    # frees up progressively for the next GEMM.
                    pss = [pp.tile([128, 512], F32, tag="ps6", bufs=HC,
                                   name=f"ps6_{s}_{ci}_{hc}")
                           for hc in range(HC)]
                    for dc in range(DC - 1):
                        for hc in range(HC):
                            nc.tensor.matmul(
                                pss[hc][:, :cw],
                                w1s[:, dc, hc * 128:(hc + 1) * 128],
                                xc[:, dc, :cw],
                                start=(dc == 0), stop=False,
                            )
                    for hc in range(HC):
                        nc.tensor.matmul(
                            pss[hc][:, :cw],
                            w1s[:, DC - 1, hc * 128:(hc + 1) * 128],
                            xc[:, DC - 1, :cw],
                            start=False, stop=True,
                        )
                        nc.scalar.activation(gc[:, hc, :cw], pss[hc][:, :cw],
                                             GELU, bias=b1col(s, hc),
                                             scale=1.0)
                else:
                    # hc-major: data resident by now; GELU per group
                    # interleaves with the next group's matmuls.
                    for hc in range(HC):
                        ps = pp.tile([128, 512], F32, tag="ps6", bufs=HC)
                        for dc in range(DC):
                            nc.tensor.matmul(
                                ps[:, :cw],
                                w1s[:, dc, hc * 128:(hc + 1) * 128],
                                xc[:, dc, :cw],
                                start=(dc == 0), stop=(dc == DC - 1),
                            )
                        nc.scalar.activation(gc[:, hc, :cw], ps[:, :cw],
                                             GELU, bias=b1col(s, hc),
                                             scale=1.0)
                return gc

            def mm2(s, ci, gc, last_chunk):
                c0, cw = slot_chunks[s][ci]
                w2s = tiles[s][1]
                # outputs grouped 3 d-chunks per DMA for bandwidth, except
                # the very last group which flushes per-d-chunk so the tail
                # pipeline drains early.
                for g2 in range(2):
                    dl, dh = 3 * g2, 3 * (g2 + 1)
                    split_out = last_chunk and g2 == 1
                    yc = yp.tile([128, 3, 512], F16, tag="y",
                                 name=f"yc_{s}_{ci}_{g2}")
                    for dc in range(dl, dh):
                        ps2 = pp.tile([128, 512], F32, tag="ps2")
                        for hc in range(HC):
                            nc.tensor.matmul(
                                ps2[:, :cw],
                                w2s[:, hc, dc * 128:(dc + 1) * 128],
                                gc[:, hc, :cw],
                                start=(hc == 0), stop=(hc == HC - 1),
                            )
                        nc.vector.tensor_scalar_add(
                            yc[:, dc - dl, :cw], ps2[:, :cw], b2col(s, dc))
                        if split_out:
                            if dc == dh - 1:
                                # final piece: split in half across both
                                # engines so the last descriptor gens and
                                # transfers (which gate the exit barrier)
                                # run in parallel
                                half = (cw // 2 + 15) & ~15
                                nc.sync.dma_start(
                                    yts_d[s].ap()[:, dc, c0:c0 + half],
                                    yc[:, dc - dl, :half])
                                nc.scalar.dma_start(
                                    yts_d[s].ap()[:, dc, c0 + half:c0 + cw],
                                    yc[:, dc - dl, half:cw])
                            else:
                                # alternate rings so descriptor generations
                                # overlap across engines
                                eng = nc.scalar if (dc % 2) else nc.sync
                                eng.dma_start(
                                    yts_d[s].ap()[:, dc, c0:c0 + cw],
                                    yc[:, dc - dl, :cw])
                    if not split_out:
                        nc.sync.dma_start(
                            yts_d[s].ap()[:, dl:dh, c0:c0 + cw],
                            yc[:, :, :cw])

            if all(len(c) == 1 for c in slot_chunks):
                gc0 = mm1(0, 0, dc_major=True)
                gc1 = mm1(1, 0, dc_major=False)
                mm2(0, 0, gc0, last_chunk=False)
                mm2(1, 0, gc1, last_chunk=True)
            else:
                # generic fallback for caps > 512 (not hit at this size)
                for s in range(EPC):
                    for ci in range(len(slot_chunks[s])):
                        gc = mm1(s, ci, dc_major=(s == 0 and ci == 0))
                        mm2(s, ci, gc,
                            last_chunk=(s == EPC - 1
                                        and ci == len(slot_chunks[s]) - 1))

    return _finalize(nc)


_NC_CACHE = {}


def _get_nc(C0, C1):
    key = (C0, C1, MM_DTYPE)
    nc = _NC_CACHE.get(key)
    if nc is None:
        nc = _build(C0, C1)
        _NC_CACHE[key] = nc
    return nc


def _cap(n):
    return int(max(64, -(-int(n) // 32) * 32))


def kernel(x, W1, b1, W2, b2, Wg, bg):
    x = np.ascontiguousarray(np.asarray(x, dtype=np.float32))
    W1 = np.asarray(W1, dtype=np.float32)
    b1 = np.asarray(b1, dtype=np.float32)
    W2 = np.asarray(W2, dtype=np.float32)
    b2 = np.asarray(b2, dtype=np.float32)
    Wg = np.asarray(Wg, dtype=np.float32)
    bg = np.asarray(bg, dtype=np.float32)

    B, N, Dx = x.shape
    assert Dx == D and W1.shape == (E, H, D)
    T = B * N
    t = x.reshape(T, D)

    # --- gate / dispatch (host): this decides the sharding ---
    logits = t @ Wg.T + bg
    idx = np.argmax(logits, axis=1)

    counts = np.bincount(idx, minlength=E)
    # slot 0 <- 8 largest experts, slot 1 <- 8 smallest
    order = np.argsort(-counts, kind="stable")
    slot_experts = [order[:NCORES], order[NCORES:]]
    C0 = _cap(counts[slot_experts[0]].max())
    C1 = _cap(counts[slot_experts[1]].max())
    caps = [C0, C1]
    nc = _get_nc(C0, C1)
    _, npdt = _mm_dt()

    tok_ids = [np.nonzero(idx == e)[0] for e in range(E)]

    # --- host-side layout prep ---
    t_mm = t.astype(npdt)
    # w1t[e, i, dc, h] = W1[e, h, dc*128+i] (partition-major, chunk, col)
    w1t_all = np.ascontiguousarray(
        W1.astype(npdt).transpose(0, 2, 1).reshape(E, DC, 128, H)
        .transpose(0, 2, 1, 3))
    w2t_all = np.ascontiguousarray(
        W2.astype(npdt).transpose(0, 2, 1).reshape(E, HC, 128, D)
        .transpose(0, 2, 1, 3))
    # b1c[e, i, hc] = b1[e, hc*128+i]
    b1c_all = np.ascontiguousarray(b1.reshape(E, HC, 128).transpose(0, 2, 1))
    b2c_all = np.ascontiguousarray(b2.reshape(E, DC, 128).transpose(0, 2, 1))

    in_maps = []
    for c in range(NCORES):
        experts = [int(slot_experts[s][c]) for s in range(EPC)]
        ballc = np.empty((128, EPC * (HC + DC)), np.float32)
        for s in range(EPC):
            ballc[:, s * (HC + DC): s * (HC + DC) + HC] = b1c_all[experts[s]]
            ballc[:, s * (HC + DC) + HC: (s + 1) * (HC + DC)] = \
                b2c_all[experts[s]]
        m = {
            "w1t": np.ascontiguousarray(w1t_all[experts]),
            "w2t": np.ascontiguousarray(w2t_all[experts]),
            "ball": ballc,
        }
        for s in range(EPC):
            C = caps[s]
            xts = np.zeros((128, DC, C), npdt)
            ids = tok_ids[experts[s]]
            n = len(ids)
            if n:
                xts[:, :, :n] = (
                    t_mm[ids].T.reshape(DC, 128, n).transpose(1, 0, 2))
            m[f"xt{s}"] = xts
        in_maps.append(m)

    res = run_bass_kernel_spmd(nc, in_maps, core_ids=list(range(NCORES)))

    out = np.empty((T, D), np.float32)
    for c in range(NCORES):
        for s in range(EPC):
            e = int(slot_experts[s][c])
            ids = tok_ids[e]
            n = len(ids)
            if n:
                yt = res.results[c][f"yt{s}"].astype(np.float32)
                out[ids] = yt.transpose(1, 0, 2).reshape(D, caps[s])[:, :n].T
    return out.reshape(B, N, D)


# revision 28
# speedup vs baseline: 1.0611x; 1.0611x over previous
"""MoE (16 experts, top-1 gate, D=H=768) Trainium2 kernel.

Strategy (expert-parallel, per the sharding hint):
  - Host computes the gate (logits argmax) — this IS the dispatch step that
    decides the sharding: tokens are routed to the core owning their expert.
  - 16 experts are sharded 2-per-core across the 8 NeuronCores. Experts are
    sorted by routed-token count: the 8 largest go in slot 0 (capacity C0),
    the 8 smallest in slot 1 (capacity C1 <= C0), so every core does the
    identical padded work and padding waste is minimized. Capacities are
    rounded to 32 columns (the matmul free dim has no 128 constraint).
  - Each core runs the two-GEMM MLP (x @ W1.T -> GELU -> @ W2.T) for its two
    experts over its routed tokens, padded to the slot capacity.
  - Host scatters per-token outputs back to the full [B, N, D] tensor.

Device kernel details:
  - Matmul operands are fp16 (PE full rate + FWL weight loads; fp32
    LDWEIGHTS cannot pipeline and halves matmul throughput; fp16 has 10
    mantissa bits -> rel err ~4e-4 end to end). PSUM accumulation is fp32,
    biases/GELU applied on fp32 PSUM. Outputs are written back fp16 (host
    converts) to halve output HBM traffic.
  - A few raw (non-Tile) dummy matmuls on an uninitialized scratch tile are
    emitted BEFORE the TileContext, so they execute during the fixed engine
    boot preamble. They keep the PE busy so its HAM clock gate (cold
    1.2 GHz -> warm 2.4 GHz after ~3.4 us of sustained activity)
    un-throttles before the real matmul stream begins.
  - DMA ring assignment: input pieces are interleaved across BOTH HWDGE
    rings (SP and ACT) in matmul consumption order — the rings share the
    ~335 GB/s HBM budget unevenly, so splitting every tensor across both
    bounds the arrival time of each phase's working set. The ACT ring gets
    only four input pieces so the scalar engine is free for GELU early.
    Outputs ride the SP ring (idle after the input fill; SWDGE/gpsimd
    output DMAs measured ~10 us late — DVE 16-bit 2-port mode starves the
    Q7 descriptor rings), except the final per-d-chunk outputs which
    alternate SP/ACT so their descriptor generation overlaps. Biases ride
    the GPSIMD SWDGE ring (issued at t~7 us, landing well before the first
    GELU, ahead of any DVE activity).
  - BIR post-processing: the first piece of w1 slot 0 (SP) and of x slot 0
    (ACT) is hoisted into 'main' ahead of the engines' entry rendezvous so
    data starts streaming ~1.5 us earlier; the unused bass const-AP memsets
    (and optionally the init barrier) are stripped.
"""

import json

import ml_dtypes
import numpy as np

import concourse.bass as bass
import concourse.mybir as mybir
import concourse.tile as tile
from concourse.bass_utils import run_bass_kernel_spmd

E = 16          # experts
D = 768         # d_model
H = 768         # d_hidden
NCORES = 8
EPC = E // NCORES   # experts (slots) per core = 2
DC = D // 128       # 6 d-chunks
HC = H // 128       # 6 h-chunks

MM_DTYPE = "f16"   # "f16" | "bf16" | "f32r"
N_WARM = 7          # warm-up matmuls (HAM un-throttle), N=512 each
HOIST_LIMITS = {"SP": 1, "Activation": 1}
STRIP_CONST_INIT = True
STRIP_INIT_BARRIER = True

F32 = mybir.dt.float32
F16 = mybir.dt.float16


def _mm_dt():
    if MM_DTYPE == "f16":
        # fp16 runs at the same PE rate as bf16 (1 col/cycle + FWL weight
        # loads) but has 10 mantissa bits instead of 7 — ~6x lower rounding
        # error. All operands here (|x| < ~6, |W| < ~0.2, GELU outputs) are
        # far inside fp16 range and accumulation is fp32 PSUM.
        return mybir.dt.float16, np.float16
    if MM_DTYPE == "bf16":
        return mybir.dt.bfloat16, ml_dtypes.bfloat16
    return mybir.dt.float32r, np.float32


def _split_multi_waits(bir):
    """Walrus (this image's build) rejects >1 sem-wait on one instruction
    ("Too many sync wait commands" on the TileContext-exit Drain). Move
    excess waits onto a chain of same-engine NoOps directly before the
    instruction — the sequencer runs them in program order, so the
    happens-after relation is preserved exactly."""
    nid = 0
    for fn in bir["functions"]:
        for blk in fn["blocks"]:
            out = []
            for ins in blk["instructions"]:
                si = ins.get("sync_info")
                waits = (si or {}).get("on_wait") or []
                if len(waits) > 1:
                    for w in waits[:-1]:
                        nid += 1
                        out.append({
                            "debug": ins.get("debug", 0),
                            "name": f"I-waitfix{nid}",
                            "opcode": "NoOp",
                            "engine": ins["engine"],
                            "ins": [],
                            "outs": [],
                            "sync_info": {"on_update": [], "on_wait": [w]},
                        })
                    si["on_wait"] = waits[-1:]
                out.append(ins)
            blk["instructions"] = out
    return bir


def _main_and_tile(bir):
    for fn in bir["functions"]:
        blocks = {b["name"]: b for b in fn["blocks"]}
        main = blocks.get("main")
        tbs = [b for n, b in blocks.items()
               if n != "main" and not n.endswith("_end")]
        if main is not None and len(tbs) == 1:
            yield main, tbs[0]


def _hoist_input_dmas(bir, limits):
    """Move the first `limits[engine]` wait-free DMACopy instructions from
    the tile block into 'main', directly before that engine's first Drain
    (the entry rendezvous arrive), so the first input pieces stream during
    part of the fixed boot preamble. Per-engine program order (hence DMA
    ring order / semaphore accounting) is preserved exactly."""
    for main, tb in _main_and_tile(bir):
        budget = dict(limits)
        hoisted, kept = [], []
        for ins in tb["instructions"]:
            si = ins.get("sync_info") or {}
            if (ins["opcode"] == "DMACopy" and not si.get("on_wait")
                    and budget.get(ins["engine"], 0) > 0):
                budget[ins["engine"]] -= 1
                hoisted.append(ins)
            else:
                kept.append(ins)
        if not hoisted:
            continue
        out, done = [], set()
        for ins in main["instructions"]:
            if ins["opcode"] == "Drain" and ins["engine"] not in done:
                done.add(ins["engine"])
                out.extend(h for h in hoisted if h["engine"] == ins["engine"])
            out.append(ins)
        if [h for h in hoisted if h["engine"] not in done]:
            continue  # unexpected engine: leave BIR unmodified for safety
        tb["instructions"] = kept
        main["instructions"] = out
    return bir


def _strip_const_init(bir, strip_barrier):
    """Remove the bass const-AP memsets (f32 0/1, bf16 1, u8 127 — unused by
    this kernel) and optionally the init all-engine barrier from 'main', so
    engines reach their first useful instruction sooner. The kernel's own
    cross-engine ordering is entirely semaphore-based (Tile-generated), so
    the barrier is not needed for correctness here."""
    for main, _tb in _main_and_tile(bir):
        out = []
        for ins in main["instructions"]:
            if ins["opcode"] == "Memset" and any(
                    "const-" in str(o.get("name", "")) for o in ins.get("outs", [])):
                continue
            if strip_barrier and (
                    "barrier_" in ins.get("name", "")
                    or (ins["opcode"] == "Drain" and _is_barrier_drain(ins))):
                continue
            out.append(ins)
        main["instructions"] = out
    return bir


def _is_barrier_drain(ins):
    si = ins.get("sync_info") or {}
    for grp in (si.get("on_wait") or []) + (si.get("on_update") or []):
        if "barrier_" in str(grp.get("ant_name", "")):
            return True
    return False


def _finalize(nc):
    bir = json.loads(nc.to_json_bytes())
    bir = _split_multi_waits(bir)
    bir = _hoist_input_dmas(bir, HOIST_LIMITS)
    if STRIP_CONST_INIT:
        bir = _strip_const_init(bir, STRIP_INIT_BARRIER)
    data = json.dumps(bir).encode()
    nc.to_json_bytes = lambda: data
    return nc


def _chunking(C):
    chunks = []
    c0 = 0
    while c0 < C:
        cw = min(512, C - c0)
        chunks.append((c0, cw))
        c0 += cw
    return chunks


def _build(C0, C1):
    """Per-core SPMD kernel: slot 0 with token capacity C0, slot 1 with C1
    (both multiples of 32). Token dim in chunks of <=512 (PSUM bank limit
    for fp32 accumulation)."""
    caps = [C0, C1]
    slot_chunks = [_chunking(C) for C in caps]

    MMDT, _ = _mm_dt()

    nc = bass.Bass("TRN2", target_bir_lowering=False, debug=False,
                   num_devices=NCORES)
    xts_d = [nc.dram_tensor(f"xt{s}", [128, DC, caps[s]], MMDT,
                            kind="ExternalInput") for s in range(EPC)]
    yts_d = [nc.dram_tensor(f"yt{s}", [128, DC, caps[s]], F16,
                            kind="ExternalOutput") for s in range(EPC)]
    w1t = nc.dram_tensor("w1t", [EPC, 128, DC, H], MMDT, kind="ExternalInput")
    w2t = nc.dram_tensor("w2t", [EPC, 128, HC, D], MMDT, kind="ExternalInput")
    # biases packed into one [128, EPC*(HC+DC)] f32 tensor: per slot s the
    # columns are [b1 cols (HC), b2 cols (DC)].
    ball = nc.dram_tensor("ball", [128, EPC * (HC + DC)], F32,
                          kind="ExternalInput")

    # scratch SBUF for the HAM warm-up matmuls (read uninitialized)
    warm_sb = nc.alloc_sbuf_tensor("warm_sb", [128, 512], MMDT)

    GELU = mybir.ActivationFunctionType.Gelu

    with tile.TileContext(nc) as tc:
        with (
            tc.tile_pool(name="xp", bufs=1) as xp,
            tc.tile_pool(name="wp", bufs=1) as wp,
            tc.tile_pool(name="gp", bufs=2) as gp,
            tc.tile_pool(name="yp", bufs=4) as yp,
            tc.tile_pool(name="bp", bufs=1) as bp,
            tc.tile_pool(name="pp", bufs=2, space="PSUM") as pp,
        ):
            # ---- HAM warm-up: matmuls on uninitialized scratch, rotated
            # through the MM2 PSUM ring (tag ps2) before MM2 ever uses it.
            # high_priority pins them to the front of the PE queue, so the
            # PE is busy (and the HAM clock gate un-throttles) while the
            # input DMAs stream.
            with tc.high_priority():
                for _ in range(N_WARM):
                    wps = pp.tile([128, 512], F32, tag="ps2", name="wps")
                    nc.tensor.matmul(wps[:, :], warm_sb.ap()[:, 0:128],
                                     warm_sb.ap(), start=True, stop=True)

            # ---- phase 1: issue ALL input DMAs. No compute-dependent wait
            # ever enters any input ring, so they stream continuously.
            tiles = []
            for s in range(EPC):
                w1s = wp.tile([128, DC, H], MMDT, tag=f"w1_{s}",
                              name=f"w1s_{s}")
                w2s = wp.tile([128, HC, D], MMDT, tag=f"w2_{s}",
                              name=f"w2s_{s}")
                xcs = [xp.tile([128, DC, 512], MMDT, tag=f"x_{s}_{ci}",
                               name=f"xc_{s}_{ci}")
                       for ci in range(len(slot_chunks[s]))]
                tiles.append((w1s, w2s, xcs))
            bt = bp.tile([128, EPC * (HC + DC)], F32, tag="b", name="bt")

            (w1s0, w2s0, xcs0), (w1s1, w2s1, xcs1) = tiles
            xc0, xc1 = xcs0[0], xcs1[0]
            cw0 = slot_chunks[0][0][1]
            cw1 = slot_chunks[1][0][1]
            # Interleave the slot-0 working set (w1s0 + x chunk0) across
            # both rings in 2-dc pieces, in matmul consumption order (the
            # dc-major first GEMM consumes (w1[dc], x[dc]) pairs). Finer
            # per-dc pieces measured ~40% lower ring throughput. The first
            # piece on each ring is hoisted into 'main' by _finalize.
            # Two HWDGE input rings (SP + ACT), every early tensor split
            # across both in dc-major consumption order. (A third SWDGE
            # input channel was measured far slower — Q7 descriptor gen
            # starts ~2.5us late and streams at ~0.1MB/us — so inputs stay
            # off gpsimd.) First piece per ring is hoisted into 'main'.
            nc.sync.dma_start(w1s0[:, 0:1], w1t.ap()[0, :, 0:1])          # SP*
            nc.scalar.dma_start(xc0[:, 0:1, :cw0],
                                xts_d[0].ap()[:, 0:1, 0:cw0])             # ACT*
            nc.sync.dma_start(xc0[:, 1:3, :cw0],
                              xts_d[0].ap()[:, 1:3, 0:cw0])
            nc.scalar.dma_start(w1s0[:, 1:3], w1t.ap()[0, :, 1:3])
            nc.sync.dma_start(w1s0[:, 3:6], w1t.ap()[0, :, 3:6])
            nc.scalar.dma_start(xc0[:, 3:6, :cw0],
                                xts_d[0].ap()[:, 3:6, 0:cw0])
            # slot-1 closure next (its first GEMM runs before either second
            # GEMM, hiding the slot-0 GELU drain and the w2 wait). Coarse
            # pieces: finer splits measurably lower ring throughput.
            nc.sync.dma_start(w1s1[:, 0:3], w1t.ap()[1, :, 0:3])
            nc.scalar.dma_start(xc1[:, :, :cw1], xts_d[1].ap()[:, :, 0:cw1])
            nc.sync.dma_start(w1s1[:, 3:6], w1t.ap()[1, :, 3:6])
            # w2 split across BOTH rings by OUTPUT d-chunks (MM2's group
            # for output dc needs w2s[:, all hc, dc*128:...]); a single
            # ring cannot deliver a w2 in time.
            nc.sync.dma_start(w2s0[:, :, 0:3 * 128], w2t.ap()[0, :, :, 0:384])
            nc.scalar.dma_start(w2s0[:, :, 3 * 128:], w2t.ap()[0, :, :, 384:768])
            nc.sync.dma_start(w2s1[:, :, 0:3 * 128], w2t.ap()[1, :, :, 0:384])
            nc.scalar.dma_start(w2s1[:, :, 3 * 128:], w2t.ap()[1, :, :, 384:768])
            # any extra x chunks (caps > 512; not hit for this problem size)
            for s, xcs in ((0, xcs0), (1, xcs1)):
                for ci, (c0, cw) in enumerate(slot_chunks[s]):
                    if ci == 0:
                        continue
                    eng = nc.sync if (ci % 2) else nc.scalar
                    eng.dma_start(xcs[ci][:, :, :cw],
                                  xts_d[s].ap()[:, :, c0:c0 + cw])
            # SWDGE ring (gpsimd, idle early): biases.
            nc.gpsimd.dma_start(bt[:, :], ball.ap())

            # ---- phase 2: compute.
            # Order: MM1(s0), MM1(s1), MM2(s0), MM2(s1) — slot 1's first
            # GEMM fills the PE bubble where slot 0's GELU drain and w2
            # arrival would otherwise stall the FIFO.
            def b1col(s, hc):
                return bt[:, s * (HC + DC) + hc: s * (HC + DC) + hc + 1]

            def b2col(s, dc):
                base = s * (HC + DC) + HC
                return bt[:, base + dc: base + dc + 1]

            def mm1(s, ci, dc_major):
                c0, cw = slot_chunks[s][ci]
                w1s = tiles[s][0]
                xc = tiles[s][2][ci]
                gc = gp.tile([128, HC, 512], MMDT, tag="g",
                             name=f"gc_{s}_{ci}")
                if dc_major:
                    # 6 accumulation groups open at once (6 PSUM banks):
                    # each arriving (w1[dc], x[dc]) pair unlocks 6 matmuls,
                    # so the PE streams during the HBM fill instead of
                    # FIFO-stalling on the first hc-group's full d sweep.
                    # The final dc pass closes groups one hc at a time with
                    # the GELU fused right after each close, so the ACT
                    # drain overlaps the remaining closes and the PSUM ring
                    # frees up progressively for the next GEMM.
                    pss = [pp.tile([128, 512], F32, tag="ps6", bufs=HC,
                                   name=f"ps6_{s}_{ci}_{hc}")
                           for hc in range(HC)]
                    for dc in range(DC - 1):
                        for hc in range(HC):
                            nc.tensor.matmul(
                                pss[hc][:, :cw],
                                w1s[:, dc, hc * 128:(hc + 1) * 128],
                                xc[:, dc, :cw],
                                start=(dc == 0), stop=False,
                            )
                    for hc in range(HC):
                        nc.tensor.matmul(
                            pss[hc][:, :cw],
                            w1s[:, DC - 1, hc * 128:(hc + 1) * 128],
                            xc[:, DC - 1, :cw],
                            start=False, stop=True,
                        )
                        nc.scalar.activation(gc[:, hc, :cw], pss[hc][:, :cw],
                                             GELU, bias=b1col(s, hc),
                                             scale=1.0)
                else:
                    # hc-major: data resident by now; GELU per group
                    # interleaves with the next group's matmuls.
                    for hc in range(HC):
                        ps = pp.tile([128, 512], F32, tag="ps6", bufs=HC)
                        for dc in range(DC):
                            nc.tensor.matmul(
                                ps[:, :cw],
                                w1s[:, dc, hc * 128:(hc + 1) * 128],
                                xc[:, dc, :cw],
                                start=(dc == 0), stop=(dc == DC - 1),
                            )
                        nc.scalar.activation(gc[:, hc, :cw], ps[:, :cw],
                                             GELU, bias=b1col(s, hc),
                                             scale=1.0)
                return gc

            def mm2(s, ci, gc, last_chunk):
                c0, cw = slot_chunks[s][ci]
                w2s = tiles[s][1]
                # outputs grouped 3 d-chunks per DMA for bandwidth, except
                # the very last group which flushes per-d-chunk so the tail
                # pipeline drains early.
                for g2 in range(2):
                    dl, dh = 3 * g2, 3 * (g2 + 1)
                    split_out = last_chunk and g2 == 1
                    yc = yp.tile([128, 3, 512], F16, tag="y",
                                 name=f"yc_{s}_{ci}_{g2}")
                    for dc in range(dl, dh):
                        ps2 = pp.tile([128, 512], F32, tag="ps2")
                        for hc in range(HC):
                            nc.tensor.matmul(
                                ps2[:, :cw],
                                w2s[:, hc, dc * 128:(dc + 1) * 128],
                                gc[:, hc, :cw],
                                start=(hc == 0), stop=(hc == HC - 1),
                            )
                        nc.vector.tensor_scalar_add(
                            yc[:, dc - dl, :cw], ps2[:, :cw], b2col(s, dc))
                        if split_out:
                            if dc == dh - 1:
                                # final piece: split in half across both
                                # engines so the last descriptor gens and
                                # transfers (which gate the exit barrier)
                                # run in parallel
                                half = (cw // 2 + 15) & ~15
                                nc.sync.dma_start(
                                    yts_d[s].ap()[:, dc, c0:c0 + half],
                                    yc[:, dc - dl, :half])
                                nc.scalar.dma_start(
                                    yts_d[s].ap()[:, dc, c0 + half:c0 + cw],
                                    yc[:, dc - dl, half:cw])
                            else:
                                # alternate rings so descriptor generations
                                # overlap across engines
                                eng = nc.scalar if (dc % 2) else nc.sync
                                eng.dma_start(
                                    yts_d[s].ap()[:, dc, c0:c0 + cw],
                                    yc[:, dc - dl, :cw])
                    if not split_out:
                        nc.sync.dma_start(
                            yts_d[s].ap()[:, dl:dh, c0:c0 + cw],
                            yc[:, :, :cw])

            if all(len(c) == 1 for c in slot_chunks):
                gc0 = mm1(0, 0, dc_major=True)
                gc1 = mm1(1, 0, dc_major=False)
                mm2(0, 0, gc0, last_chunk=False)
                mm2(1, 0, gc1, last_chunk=True)
            else:
                # generic fallback for caps > 512 (not hit at this size)
                for s in range(EPC):
                    for ci in range(len(slot_chunks[s])):
                        gc = mm1(s, ci, dc_major=(s == 0 and ci == 0))
                        mm2(s, ci, gc,
                            last_chunk=(s == EPC - 1
                                        and ci == len(slot_chunks[s]) - 1))

    return _finalize(nc)


_NC_CACHE = {}


def _get_nc(C0, C1):
    key = (C0, C1, MM_DTYPE)
    nc = _NC_CACHE.get(key)
    if nc is None:
        nc = _build(C0, C1)
        _NC_CACHE[key] = nc
    return nc


def _cap(n):
    return int(max(64, -(-int(n) // 32) * 32))


def kernel(x, W1, b1, W2, b2, Wg, bg):
    x = np.ascontiguousarray(np.asarray(x, dtype=np.float32))
    W1 = np.asarray(W1, dtype=np.float32)
    b1 = np.asarray(b1, dtype=np.float32)
    W2 = np.asarray(W2, dtype=np.float32)
    b2 = np.asarray(b2, dtype=np.float32)
    Wg = np.asarray(Wg, dtype=np.float32)
    bg = np.asarray(bg, dtype=np.float32)

    B, N, Dx = x.shape
    assert Dx == D and W1.shape == (E, H, D)
    T = B * N
    t = x.reshape(T, D)

    # --- gate / dispatch (host): this decides the sharding ---
    logits = t @ Wg.T + bg
    idx = np.argmax(logits, axis=1)

    counts = np.bincount(idx, minlength=E)
    # slot 0 <- 8 largest experts, slot 1 <- 8 smallest
    order = np.argsort(-counts, kind="stable")
    slot_experts = [order[:NCORES], order[NCORES:]]
    C0 = _cap(counts[slot_experts[0]].max())
    C1 = _cap(counts[slot_experts[1]].max())
    caps = [C0, C1]
    nc = _get_nc(C0, C1)
    _, npdt = _mm_dt()

    tok_ids = [np.nonzero(idx == e)[0] for e in range(E)]

    # --- host-side layout prep ---
    t_mm = t.astype(npdt)
    # w1t[e, i, dc, h] = W1[e, h, dc*128+i] (partition-major, chunk, col)
    w1t_all = np.ascontiguousarray(
        W1.astype(npdt).transpose(0, 2, 1).reshape(E, DC, 128, H)
        .transpose(0, 2, 1, 3))
    w2t_all = np.ascontiguousarray(
        W2.astype(npdt).transpose(0, 2, 1).reshape(E, HC, 128, D)
        .transpose(0, 2, 1, 3))
    # b1c[e, i, hc] = b1[e, hc*128+i]
    b1c_all = np.ascontiguousarray(b1.reshape(E, HC, 128).transpose(0, 2, 1))
    b2c_all = np.ascontiguousarray(b2.reshape(E, DC, 128).transpose(0, 2, 1))

    in_maps = []
    for c in range(NCORES):
        experts = [int(slot_experts[s][c]) for s in range(EPC)]
        ballc = np.empty((128, EPC * (HC + DC)), np.float32)
        for s in range(EPC):
            ballc[:, s * (HC + DC): s * (HC + DC) + HC] = b1c_all[experts[s]]
            ballc[:, s * (HC + DC) + HC: (s + 1) * (HC + DC)] = \
                b2c_all[experts[s]]
        m = {
            "w1t": np.ascontiguousarray(w1t_all[experts]),
            "w2t": np.ascontiguousarray(w2t_all[experts]),
            "ball": ballc,
        }
        for s in range(EPC):
            C = caps[s]
            xts = np.zeros((128, DC, C), npdt)
            ids = tok_ids[experts[s]]
            n = len(ids)
            if n:
                xts[:, :, :n] = (
                    t_mm[ids].T.reshape(DC, 128, n).transpose(1, 0, 2))
            m[f"xt{s}"] = xts
        in_maps.append(m)

    res = run_bass_kernel_spmd(nc, in_maps, core_ids=list(range(NCORES)))

    out = np.empty((T, D), np.float32)
    for c in range(NCORES):
        for s in range(EPC):
            e = int(slot_experts[s][c])
            ids = tok_ids[e]
            n = len(ids)
            if n:
                yt = res.results[c][f"yt{s}"].astype(np.float32)
                out[ids] = yt.transpose(1, 0, 2).reshape(D, caps[s])[:, :n].T
    return out.reshape(B, N, D)


# revision 32
# speedup vs baseline: 1.0748x; 1.0130x over previous
"""MoE (16 experts, top-1 gate, D=H=768) Trainium2 kernel.

Strategy (expert-parallel, per the sharding hint):
  - Host computes the gate (logits argmax) — this IS the dispatch step that
    decides the sharding: tokens are routed to the core owning their expert.
  - 16 experts are sharded 2-per-core across the 8 NeuronCores. Experts are
    sorted by routed-token count: the 8 largest go in slot 0 (capacity C0),
    the 8 smallest in slot 1 (capacity C1 <= C0), so every core does the
    identical padded work and padding waste is minimized. Capacities are
    rounded to 32 columns (the matmul free dim has no 128 constraint).
  - Each core runs the two-GEMM MLP (x @ W1.T -> GELU -> @ W2.T) for its two
    experts over its routed tokens, padded to the slot capacity.
  - Host scatters per-token outputs back to the full [B, N, D] tensor.

Device kernel details:
  - Matmul operands are fp16 (PE full rate + FWL weight loads; fp32
    LDWEIGHTS cannot pipeline and halves matmul throughput; fp16 has 10
    mantissa bits -> rel err ~4e-4 end to end). PSUM accumulation is fp32,
    biases/GELU applied on fp32 PSUM. Outputs are written back fp16 (host
    converts) to halve output HBM traffic.
  - A few raw (non-Tile) dummy matmuls on an uninitialized scratch tile are
    emitted BEFORE the TileContext, so they execute during the fixed engine
    boot preamble. They keep the PE busy so its HAM clock gate (cold
    1.2 GHz -> warm 2.4 GHz after ~3.4 us of sustained activity)
    un-throttles before the real matmul stream begins.
  - DMA ring assignment: input pieces are interleaved across BOTH HWDGE
    rings (SP and ACT) in matmul consumption order — the rings share the
    ~335 GB/s HBM budget unevenly, so splitting every tensor across both
    bounds the arrival time of each phase's working set. The ACT ring gets
    only four input pieces so the scalar engine is free for GELU early.
    Outputs ride the SP ring (idle after the input fill; SWDGE/gpsimd
    output DMAs measured ~10 us late — DVE 16-bit 2-port mode starves the
    Q7 descriptor rings), except the final per-d-chunk outputs which
    alternate SP/ACT so their descriptor generation overlaps. Biases ride
    the GPSIMD SWDGE ring (issued at t~7 us, landing well before the first
    GELU, ahead of any DVE activity).
  - BIR post-processing: the first piece of w1 slot 0 (SP) and of x slot 0
    (ACT) is hoisted into 'main' ahead of the engines' entry rendezvous so
    data starts streaming ~1.5 us earlier; the unused bass const-AP memsets
    (and optionally the init barrier) are stripped.
"""

import json

import ml_dtypes
import numpy as np

import concourse.bass as bass
import concourse.mybir as mybir
import concourse.tile as tile
from concourse.bass_utils import run_bass_kernel_spmd

E = 16          # experts
D = 768         # d_model
H = 768         # d_hidden
NCORES = 8
EPC = E // NCORES   # experts (slots) per core = 2
DC = D // 128       # 6 d-chunks
HC = H // 128       # 6 h-chunks

MM_DTYPE = "f16"   # "f16" | "bf16" | "f32r"
N_WARM = 7          # warm-up matmuls (HAM un-throttle), N=512 each
HOIST_LIMITS = {"SP": 2, "Activation": 2}
STRIP_CONST_INIT = True
STRIP_INIT_BARRIER = True

F32 = mybir.dt.float32
F16 = mybir.dt.float16


def _mm_dt():
    if MM_DTYPE == "f16":
        # fp16 runs at the same PE rate as bf16 (1 col/cycle + FWL weight
        # loads) but has 10 mantissa bits instead of 7 — ~6x lower rounding
        # error. All operands here (|x| < ~6, |W| < ~0.2, GELU outputs) are
        # far inside fp16 range and accumulation is fp32 PSUM.
        return mybir.dt.float16, np.float16
    if MM_DTYPE == "bf16":
        return mybir.dt.bfloat16, ml_dtypes.bfloat16
    return mybir.dt.float32r, np.float32


def _split_multi_waits(bir):
    """Walrus (this image's build) rejects >1 sem-wait on one instruction
    ("Too many sync wait commands" on the TileContext-exit Drain). Move
    excess waits onto a chain of same-engine NoOps directly before the
    instruction — the sequencer runs them in program order, so the
    happens-after relation is preserved exactly."""
    nid = 0
    for fn in bir["functions"]:
        for blk in fn["blocks"]:
            out = []
            for ins in blk["instructions"]:
                si = ins.get("sync_info")
                waits = (si or {}).get("on_wait") or []
                if len(waits) > 1:
                    for w in waits[:-1]:
                        nid += 1
                        out.append({
                            "debug": ins.get("debug", 0),
                            "name": f"I-waitfix{nid}",
                            "opcode": "NoOp",
                            "engine": ins["engine"],
                            "ins": [],
                            "outs": [],
                            "sync_info": {"on_update": [], "on_wait": [w]},
                        })
                    si["on_wait"] = waits[-1:]
                out.append(ins)
            blk["instructions"] = out
    return bir


def _main_and_tile(bir):
    for fn in bir["functions"]:
        blocks = {b["name"]: b for b in fn["blocks"]}
        main = blocks.get("main")
        tbs = [b for n, b in blocks.items()
               if n != "main" and not n.endswith("_end")]
        if main is not None and len(tbs) == 1:
            yield main, tbs[0]


def _hoist_input_dmas(bir, limits):
    """Move the first `limits[engine]` wait-free DMACopy instructions from
    the tile block into 'main', directly before that engine's first Drain
    (the entry rendezvous arrive), so the first input pieces stream during
    part of the fixed boot preamble. Per-engine program order (hence DMA
    ring order / semaphore accounting) is preserved exactly."""
    for main, tb in _main_and_tile(bir):
        budget = dict(limits)
        hoisted, kept = [], []
        for ins in tb["instructions"]:
            si = ins.get("sync_info") or {}
            if (ins["opcode"] == "DMACopy" and not si.get("on_wait")
                    and budget.get(ins["engine"], 0) > 0):
                budget[ins["engine"]] -= 1
                hoisted.append(ins)
            else:
                kept.append(ins)
        if not hoisted:
            continue
        out, done = [], set()
        for ins in main["instructions"]:
            if ins["opcode"] == "Drain" and ins["engine"] not in done:
                done.add(ins["engine"])
                out.extend(h for h in hoisted if h["engine"] == ins["engine"])
            out.append(ins)
        if [h for h in hoisted if h["engine"] not in done]:
            continue  # unexpected engine: leave BIR unmodified for safety
        tb["instructions"] = kept
        main["instructions"] = out
    return bir


def _strip_const_init(bir, strip_barrier):
    """Remove the bass const-AP memsets (f32 0/1, bf16 1, u8 127 — unused by
    this kernel) and optionally the init all-engine barrier from 'main', so
    engines reach their first useful instruction sooner. The kernel's own
    cross-engine ordering is entirely semaphore-based (Tile-generated), so
    the barrier is not needed for correctness here."""
    for main, _tb in _main_and_tile(bir):
        out = []
        for ins in main["instructions"]:
            if ins["opcode"] == "Memset" and any(
                    "const-" in str(o.get("name", "")) for o in ins.get("outs", [])):
                continue
            if strip_barrier and (
                    "barrier_" in ins.get("name", "")
                    or (ins["opcode"] == "Drain" and _is_barrier_drain(ins))):
                continue
            out.append(ins)
        main["instructions"] = out
    return bir


def _is_barrier_drain(ins):
    si = ins.get("sync_info") or {}
    for grp in (si.get("on_wait") or []) + (si.get("on_update") or []):
        if "barrier_" in str(grp.get("ant_name", "")):
            return True
    return False


def _finalize(nc):
    bir = json.loads(nc.to_json_bytes())
    bir = _split_multi_waits(bir)
    bir = _hoist_input_dmas(bir, HOIST_LIMITS)
    if STRIP_CONST_INIT:
        bir = _strip_const_init(bir, STRIP_INIT_BARRIER)
    data = json.dumps(bir).encode()
    nc.to_json_bytes = lambda: data
    return nc


def _chunking(C):
    chunks = []
    c0 = 0
    while c0 < C:
        cw = min(512, C - c0)
        chunks.append((c0, cw))
        c0 += cw
    return chunks


def _build(C0, C1):
    """Per-core SPMD kernel: slot 0 with token capacity C0, slot 1 with C1
    (both multiples of 32). Token dim in chunks of <=512 (PSUM bank limit
    for fp32 accumulation)."""
    caps = [C0, C1]
    slot_chunks = [_chunking(C) for C in caps]

    MMDT, _ = _mm_dt()

    nc = bass.Bass("TRN2", target_bir_lowering=False, debug=False,
                   num_devices=NCORES)
    xts_d = [nc.dram_tensor(f"xt{s}", [128, DC, caps[s]], MMDT,
                            kind="ExternalInput") for s in range(EPC)]
    yts_d = [nc.dram_tensor(f"yt{s}", [128, DC, caps[s]], F16,
                            kind="ExternalOutput") for s in range(EPC)]
    w1t = nc.dram_tensor("w1t", [EPC, 128, DC, H], MMDT, kind="ExternalInput")
    w2t = nc.dram_tensor("w2t", [EPC, 128, HC, D], MMDT, kind="ExternalInput")
    # biases packed into one [128, EPC*(HC+DC)] f32 tensor: per slot s the
    # columns are [b1 cols (HC), b2 cols (DC)].
    ball = nc.dram_tensor("ball", [128, EPC * (HC + DC)], F32,
                          kind="ExternalInput")

    # scratch SBUF for the HAM warm-up matmuls (read uninitialized)
    warm_sb = nc.alloc_sbuf_tensor("warm_sb", [128, 512], MMDT)

    GELU = mybir.ActivationFunctionType.Gelu

    with tile.TileContext(nc) as tc:
        with (
            tc.tile_pool(name="xp", bufs=1) as xp,
            tc.tile_pool(name="wp", bufs=1) as wp,
            tc.tile_pool(name="gp", bufs=2) as gp,
            tc.tile_pool(name="yp", bufs=4) as yp,
            tc.tile_pool(name="bp", bufs=1) as bp,
            tc.tile_pool(name="pp", bufs=2, space="PSUM") as pp,
        ):
            # ---- HAM warm-up: matmuls on uninitialized scratch, rotated
            # through the MM2 PSUM ring (tag ps2) before MM2 ever uses it.
            # high_priority pins them to the front of the PE queue, so the
            # PE is busy (and the HAM clock gate un-throttles) while the
            # input DMAs stream.
            with tc.high_priority():
                for _ in range(N_WARM):
                    wps = pp.tile([128, 512], F32, tag="ps2", name="wps")
                    nc.tensor.matmul(wps[:, :], warm_sb.ap()[:, 0:128],
                                     warm_sb.ap(), start=True, stop=True)

            # ---- phase 1: issue ALL input DMAs. No compute-dependent wait
            # ever enters any input ring, so they stream continuously.
            tiles = []
            for s in range(EPC):
                w1s = wp.tile([128, DC, H], MMDT, tag=f"w1_{s}",
                              name=f"w1s_{s}")
                w2s = wp.tile([128, HC, D], MMDT, tag=f"w2_{s}",
                              name=f"w2s_{s}")
                xcs = [xp.tile([128, DC, 512], MMDT, tag=f"x_{s}_{ci}",
                               name=f"xc_{s}_{ci}")
                       for ci in range(len(slot_chunks[s]))]
                tiles.append((w1s, w2s, xcs))
            bt = bp.tile([128, EPC * (HC + DC)], F32, tag="b", name="bt")

            (w1s0, w2s0, xcs0), (w1s1, w2s1, xcs1) = tiles
            xc0, xc1 = xcs0[0], xcs1[0]
            cw0 = slot_chunks[0][0][1]
            cw1 = slot_chunks[1][0][1]
            # Interleave the slot-0 working set (w1s0 + x chunk0) across
            # both rings in 2-dc pieces, in matmul consumption order (the
            # dc-major first GEMM consumes (w1[dc], x[dc]) pairs). Finer
            # per-dc pieces measured ~40% lower ring throughput. The first
            # piece on each ring is hoisted into 'main' by _finalize.
            # Two HWDGE input rings (SP + ACT), every early tensor split
            # across both in dc-major consumption order. (A third SWDGE
            # input channel was measured far slower — Q7 descriptor gen
            # starts ~2.5us late and streams at ~0.1MB/us — so inputs stay
            # off gpsimd.) First piece per ring is hoisted into 'main'.
            nc.sync.dma_start(w1s0[:, 0:1], w1t.ap()[0, :, 0:1])          # SP*
            nc.scalar.dma_start(xc0[:, 0:1, :cw0],
                                xts_d[0].ap()[:, 0:1, 0:cw0])             # ACT*
            nc.sync.dma_start(xc0[:, 1:3, :cw0],
                              xts_d[0].ap()[:, 1:3, 0:cw0])
            nc.scalar.dma_start(w1s0[:, 1:3], w1t.ap()[0, :, 1:3])
            nc.sync.dma_start(w1s0[:, 3:6], w1t.ap()[0, :, 3:6])
            nc.scalar.dma_start(xc0[:, 3:6, :cw0],
                                xts_d[0].ap()[:, 3:6, 0:cw0])
            # slot-1 closure next (its first GEMM runs before either second
            # GEMM, hiding the slot-0 GELU drain and the w2 wait). Coarse
            # pieces: finer splits measurably lower ring throughput.
            nc.sync.dma_start(w1s1[:, 0:3], w1t.ap()[1, :, 0:3])
            nc.scalar.dma_start(xc1[:, :, :cw1], xts_d[1].ap()[:, :, 0:cw1])
            nc.sync.dma_start(w1s1[:, 3:6], w1t.ap()[1, :, 3:6])
            # w2 split across BOTH rings by OUTPUT d-chunks (MM2's group
            # for output dc needs w2s[:, all hc, dc*128:...]); a single
            # ring cannot deliver a w2 in time.
            nc.sync.dma_start(w2s0[:, :, 0:3 * 128], w2t.ap()[0, :, :, 0:384])
            nc.scalar.dma_start(w2s0[:, :, 3 * 128:], w2t.ap()[0, :, :, 384:768])
            nc.sync.dma_start(w2s1[:, :, 0:3 * 128], w2t.ap()[1, :, :, 0:384])
            nc.scalar.dma_start(w2s1[:, :, 3 * 128:], w2t.ap()[1, :, :, 384:768])
            # any extra x chunks (caps > 512; not hit for this problem size)
            for s, xcs in ((0, xcs0), (1, xcs1)):
                for ci, (c0, cw) in enumerate(slot_chunks[s]):
                    if ci == 0:
                        continue
                    eng = nc.sync if (ci % 2) else nc.scalar
                    eng.dma_start(xcs[ci][:, :, :cw],
                                  xts_d[s].ap()[:, :, c0:c0 + cw])
            # SWDGE ring (gpsimd, idle early): biases.
            nc.gpsimd.dma_start(bt[:, :], ball.ap())

            # ---- phase 2: compute.
            # Order: MM1(s0), MM1(s1), MM2(s0), MM2(s1) — slot 1's first
            # GEMM fills the PE bubble where slot 0's GELU drain and w2
            # arrival would otherwise stall the FIFO.
            def b1col(s, hc):
                return bt[:, s * (HC + DC) + hc: s * (HC + DC) + hc + 1]

            def b2col(s, dc):
                base = s * (HC + DC) + HC
                return bt[:, base + dc: base + dc + 1]

            def mm1(s, ci, dc_major, borrow_first=False):
                c0, cw = slot_chunks[s][ci]
                w1s = tiles[s][0]
                xc = tiles[s][2][ci]
                gc = gp.tile([128, HC, 512], MMDT, tag="g",
                             name=f"gc_{s}_{ci}")
                if dc_major:
                    # 6 accumulation groups open at once (6 PSUM banks):
                    # each arriving (w1[dc], x[dc]) pair unlocks 6 matmuls,
                    # so the PE streams during the HBM fill instead of
                    # FIFO-stalling on the first hc-group's full d sweep.
                    # The final dc pass closes groups one hc at a time with
                    # the GELU fused right after each close, so the ACT
                    # drain overlaps the remaining closes and the PSUM ring
                    # frees up progressively for the next GEMM.
                    pss = [pp.tile([128, 512], F32, tag="ps6", bufs=HC,
                                   name=f"ps6_{s}_{ci}_{hc}")
                           for hc in range(HC)]
                    for dc in range(DC - 1):
                        for hc in range(HC):
                            nc.tensor.matmul(
                                pss[hc][:, :cw],
                                w1s[:, dc, hc * 128:(hc + 1) * 128],
                                xc[:, dc, :cw],
                                start=(dc == 0), stop=False,
                            )
                    for hc in range(HC):
                        nc.tensor.matmul(
                            pss[hc][:, :cw],
                            w1s[:, DC - 1, hc * 128:(hc + 1) * 128],
                            xc[:, DC - 1, :cw],
                            start=False, stop=True,
                        )
                        nc.scalar.activation(gc[:, hc, :cw], pss[hc][:, :cw],
                                             GELU, bias=b1col(s, hc),
                                             scale=1.0)
                else:
                    # hc-major: data resident by now; GELU per group
                    # interleaves with the next group's matmuls. The first
                    # group can borrow a bank from the (idle) ps2 ring so
                    # it does not wait for the previous GEMM's first GELU
                    # to free the ps6 ring.
                    for hc in range(HC):
                        if hc == 0 and borrow_first:
                            ps = pp.tile([128, 512], F32, tag="ps2")
                        else:
                            ps = pp.tile([128, 512], F32, tag="ps6", bufs=HC)
                        for dc in range(DC):
                            nc.tensor.matmul(
                                ps[:, :cw],
                                w1s[:, dc, hc * 128:(hc + 1) * 128],
                                xc[:, dc, :cw],
                                start=(dc == 0), stop=(dc == DC - 1),
                            )
                        nc.scalar.activation(gc[:, hc, :cw], ps[:, :cw],
                                             GELU, bias=b1col(s, hc),
                                             scale=1.0)
                return gc

            def mm2(s, ci, gc, last_chunk):
                c0, cw = slot_chunks[s][ci]
                w2s = tiles[s][1]
                # outputs grouped 3 d-chunks per DMA for bandwidth, except
                # the very last group which flushes per-d-chunk so the tail
                # pipeline drains early.
                for g2 in range(2):
                    dl, dh = 3 * g2, 3 * (g2 + 1)
                    split_out = last_chunk and g2 == 1
                    yc = yp.tile([128, 3, 512], F16, tag="y",
                                 name=f"yc_{s}_{ci}_{g2}")
                    for dc in range(dl, dh):
                        ps2 = pp.tile([128, 512], F32, tag="ps2")
                        for hc in range(HC):
                            nc.tensor.matmul(
                                ps2[:, :cw],
                                w2s[:, hc, dc * 128:(dc + 1) * 128],
                                gc[:, hc, :cw],
                                start=(hc == 0), stop=(hc == HC - 1),
                            )
                        nc.vector.tensor_scalar_add(
                            yc[:, dc - dl, :cw], ps2[:, :cw], b2col(s, dc))
                        if split_out:
                            if dc == dh - 1:
                                # final piece: split in half across both
                                # engines so the last descriptor gens and
                                # transfers (which gate the exit barrier)
                                # run in parallel
                                half = (cw // 2 + 15) & ~15
                                nc.sync.dma_start(
                                    yts_d[s].ap()[:, dc, c0:c0 + half],
                                    yc[:, dc - dl, :half])
                                nc.scalar.dma_start(
                                    yts_d[s].ap()[:, dc, c0 + half:c0 + cw],
                                    yc[:, dc - dl, half:cw])
                            else:
                                # alternate rings so descriptor generations
                                # overlap across engines
                                eng = nc.scalar if (dc % 2) else nc.sync
                                eng.dma_start(
                                    yts_d[s].ap()[:, dc, c0:c0 + cw],
                                    yc[:, dc - dl, :cw])
                    if not split_out:
                        nc.sync.dma_start(
                            yts_d[s].ap()[:, dl:dh, c0:c0 + cw],
                            yc[:, :, :cw])

            if all(len(c) == 1 for c in slot_chunks):
                gc0 = mm1(0, 0, dc_major=True)
                gc1 = mm1(1, 0, dc_major=False, borrow_first=True)
                mm2(0, 0, gc0, last_chunk=False)
                mm2(1, 0, gc1, last_chunk=True)
            else:
                # generic fallback for caps > 512 (not hit at this size)
                for s in range(EPC):
                    for ci in range(len(slot_chunks[s])):
                        gc = mm1(s, ci, dc_major=(s == 0 and ci == 0))
                        mm2(s, ci, gc,
                            last_chunk=(s == EPC - 1
                                        and ci == len(slot_chunks[s]) - 1))

    return _finalize(nc)


_NC_CACHE = {}


def _get_nc(C0, C1):
    key = (C0, C1, MM_DTYPE)
    nc = _NC_CACHE.get(key)
    if nc is None:
        nc = _build(C0, C1)
        _NC_CACHE[key] = nc
    return nc


def _cap(n):
    return int(max(64, -(-int(n) // 32) * 32))


def kernel(x, W1, b1, W2, b2, Wg, bg):
    x = np.ascontiguousarray(np.asarray(x, dtype=np.float32))
    W1 = np.asarray(W1, dtype=np.float32)
    b1 = np.asarray(b1, dtype=np.float32)
    W2 = np.asarray(W2, dtype=np.float32)
    b2 = np.asarray(b2, dtype=np.float32)
    Wg = np.asarray(Wg, dtype=np.float32)
    bg = np.asarray(bg, dtype=np.float32)

    B, N, Dx = x.shape
    assert Dx == D and W1.shape == (E, H, D)
    T = B * N
    t = x.reshape(T, D)

    # --- gate / dispatch (host): this decides the sharding ---
    logits = t @ Wg.T + bg
    idx = np.argmax(logits, axis=1)

    counts = np.bincount(idx, minlength=E)
    # slot 0 <- 8 largest experts, slot 1 <- 8 smallest
    order = np.argsort(-counts, kind="stable")
    slot_experts = [order[:NCORES], order[NCORES:]]
    C0 = _cap(counts[slot_experts[0]].max())
    C1 = _cap(counts[slot_experts[1]].max())
    caps = [C0, C1]
    nc = _get_nc(C0, C1)
    _, npdt = _mm_dt()

    tok_ids = [np.nonzero(idx == e)[0] for e in range(E)]

    # --- host-side layout prep ---
    t_mm = t.astype(npdt)
    # w1t[e, i, dc, h] = W1[e, h, dc*128+i] (partition-major, chunk, col)
    w1t_all = np.ascontiguousarray(
        W1.astype(npdt).transpose(0, 2, 1).reshape(E, DC, 128, H)
        .transpose(0, 2, 1, 3))
    w2t_all = np.ascontiguousarray(
        W2.astype(npdt).transpose(0, 2, 1).reshape(E, HC, 128, D)
        .transpose(0, 2, 1, 3))
    # b1c[e, i, hc] = b1[e, hc*128+i]
    b1c_all = np.ascontiguousarray(b1.reshape(E, HC, 128).transpose(0, 2, 1))
    b2c_all = np.ascontiguousarray(b2.reshape(E, DC, 128).transpose(0, 2, 1))

    in_maps = []
    for c in range(NCORES):
        experts = [int(slot_experts[s][c]) for s in range(EPC)]
        ballc = np.empty((128, EPC * (HC + DC)), np.float32)
        for s in range(EPC):
            ballc[:, s * (HC + DC): s * (HC + DC) + HC] = b1c_all[experts[s]]
            ballc[:, s * (HC + DC) + HC: (s + 1) * (HC + DC)] = \
                b2c_all[experts[s]]
        m = {
            "w1t": np.ascontiguousarray(w1t_all[experts]),
            "w2t": np.ascontiguousarray(w2t_all[experts]),
            "ball": ballc,
        }
        for s in range(EPC):
            C = caps[s]
            xts = np.zeros((128, DC, C), npdt)
            ids = tok_ids[experts[s]]
            n = len(ids)
            if n:
                xts[:, :, :n] = (
                    t_mm[ids].T.reshape(DC, 128, n).transpose(1, 0, 2))
            m[f"xt{s}"] = xts
        in_maps.append(m)

    res = run_bass_kernel_spmd(nc, in_maps, core_ids=list(range(NCORES)))

    out = np.empty((T, D), np.float32)
    for c in range(NCORES):
        for s in range(EPC):
            e = int(slot_experts[s][c])
            ids = tok_ids[e]
            n = len(ids)
            if n:
                yt = res.results[c][f"yt{s}"].astype(np.float32)
                out[ids] = yt.transpose(1, 0, 2).reshape(D, caps[s])[:, :n].T
    return out.reshape(B, N, D)
